# revision 19
# baseline (speedup 1.0000x reference)
"""Segmented irrep linear (irreps 128x0e+128x1o+128x2e) on 8 TRN2 NeuronCores.

Reference op, per node n (100000 nodes, feature dim 1152):
  y[n, off_l + u*d_l + i] = pw * sum_u' x[n, off_l + u'*d_l + i] * W_l[u', u]
with pw = 128^-0.5, and bias b added on the l=0 (scalar, d=1) output slice.

Strategy (memory-bound: HBM traffic dominates):
  - Data-parallel over nodes: pad to 8 * 12544 rows, one shard per core.
  - All device I/O in fp16 (halves HBM bytes vs fp32; matmul accumulates in
    fp32 PSUM so the end-to-end max error is ~4e-4, far inside the 2e-2
    gate). Host casts x -> fp16 planes and upcasts y afterwards.
  - Host-side layout prep (cheap, off-device): weights pre-scaled by pw and
    packed [u, (l,v)]; x repacked into nine [u=128, n] planes, one per
    (l, i) = (irrep segment, m-component). Output comes back in the same
    plane-major layout [9, 128, n] and the host transposes it back.
  - Device (per core): stream 1024-node blocks. Per 512-node chunk, run
    nine w-stationary matmuls psO[v, n] = W_l^T @ x_(l,i)[u, n] with 512
    moving columns each (one PSUM bank per matmul, long back-to-back PE
    bursts), add the bias on the l=0 plane via a per-partition
    tensor_scalar add, and drain PSUM -> SBUF fp16 with DVE/ACT copies.
    Input DMAs issue on the SP HWDGE ring and output DMAs on the ACT ring;
    both directions are 2KB-contiguous runs per partition.

Measured on trn2 (8 cores, core-0 neuron-profile): ~321 us at fp32 I/O
(DMA-bound at the HBM roofline), ~175 us with fp16 I/O.
"""

import numpy as np

import concourse.bass as bass
import concourse.tile as tile
from concourse import bacc, mybir
from concourse.bass_utils import run_bass_kernel_spmd

N_CORES = 8
N_NODES = 100000
DIM = 1152
IRREPS = [(128, 1), (128, 3), (128, 5)]
SEG_OFF_X = [0, 128, 512]
PW = 1.0 / np.sqrt(128.0)

TILE_P = 128
TILES_PER_CORE = 98
SHARD = TILES_PER_CORE * TILE_P  # 12544
PAD_NODES = N_CORES * SHARD  # 100352
NB = 1024  # nodes per DMA block (2KB fp16 runs x 9 planes per direction)
CHUNK = 512  # moving columns per matmul = one PSUM bank of fp32

# plane order: (l, i) = (irrep segment, m-component), grouped by l so
# consecutive matmuls share the same stationary weight tile
BLOCKS = [(l, i) for l, (mul, d) in enumerate(IRREPS) for i in range(d)]

_cache = {}


def _build(shard=SHARD, nb_size=NB):
    nc = bacc.Bacc(
        "TRN2", target_bir_lowering=False, debug=False, num_devices=N_CORES
    )
    f32 = mybir.dt.float32
    f16 = mybir.dt.float16
    xt_d = nc.dram_tensor("xt", [9, 128, shard], f16, kind="ExternalInput")
    w_d = nc.dram_tensor("w", [128, 384], f16, kind="ExternalInput")
    bias_d = nc.dram_tensor("bias", [128, 1], f32, kind="ExternalInput")
    y_d = nc.dram_tensor("y", [9, 128, shard], f16, kind="ExternalOutput")

    xt_v = xt_d.ap().rearrange("b u n -> u b n")
    y_v = y_d.ap().rearrange("b v n -> v b n")

    with tile.TileContext(nc) as tc:
        with (
            tc.tile_pool(name="const", bufs=1) as const_pool,
            tc.tile_pool(name="xin", bufs=4) as x_pool,
            tc.tile_pool(name="out", bufs=4) as out_pool,
            tc.tile_pool(name="psO", bufs=8, space=bass.MemorySpace.PSUM) as ps_pool,
        ):
            w_sb = const_pool.tile([128, 384], f16)
            nc.sync.dma_start(w_sb[:], w_d.ap())
            bias_sb = const_pool.tile([128, 1], f32)
            nc.sync.dma_start(bias_sb[:], bias_d.ap())

            # node-block sizes: small blocks first so compute starts early,
            # small blocks last so the compute-paced drain at the end is
            # fine-grained
            head = [128, 128, 256, 512]
            tail = [512, 256, 256, 128, 128]
            rem = shard - sum(head) - sum(tail)
            sizes = list(head)
            while rem > 0:
                m = min(nb_size, rem)
                sizes.append(m)
                rem -= m
            sizes += tail

            n0 = 0
            for nb in sizes:
                x_sb = x_pool.tile([TILE_P, 9, nb_size], f16, tag="x")
                nc.sync.dma_start(x_sb[:, :, :nb], xt_v[:, :, n0:n0 + nb])
                out_sb = out_pool.tile([TILE_P, 9, nb_size], f16, tag="out")

                for c0 in range(0, nb, CHUNK):
                    cw = min(CHUNK, nb - c0)
                    for p, (l, i) in enumerate(BLOCKS):
                        ps = ps_pool.tile([128, CHUNK], f32, tag="ps")
                        nc.tensor.matmul(
                            ps[:, :cw],
                            w_sb[:, l * 128:(l + 1) * 128],
                            x_sb[:, p, c0:c0 + cw],
                            start=True, stop=True,
                        )
                        dst = out_sb[:, p, c0:c0 + cw]
                        if p == 0:
                            nc.vector.tensor_scalar_add(dst, ps[:, :cw], bias_sb[:])
                        elif p <= 3:
                            nc.vector.tensor_copy(dst, ps[:, :cw])
                        else:
                            nc.scalar.copy(dst, ps[:, :cw])

                # out-DMAs on the ACT HWDGE ring: separate FIFO from the
                # input stream on the SP ring, so a not-yet-ready output
                # can't head-of-line-block input prefetch
                nc.scalar.dma_start(
                    y_v[:, :, n0:n0 + nb], out_sb[:, :, :nb]
                )
                n0 += nb

    nc.compile()
    return nc


def _host_prep(w, b):
    w = np.asarray(w, dtype=np.float32)
    b = np.asarray(b, dtype=np.float32)
    w_pack = np.empty((128, 384), dtype=np.float16)
    off = 0
    for l, (mul, d) in enumerate(IRREPS):
        W = w[off:off + mul * mul].reshape(mul, mul)  # [u, v]
        w_pack[:, l * 128:(l + 1) * 128] = (PW * W).astype(np.float16)
        off += mul * mul
    bias_col = b.reshape(128, 1).astype(np.float32)
    return w_pack, bias_col


def _ensure_ntff_hook():
    """The agent image's antenv lacks axon_hooks; synthesize it from the
    boot package's ctypes NTFF hook so trace=True works."""
    import sys
    import types

    if "antenv.axon_hooks" in sys.modules:
        return
    try:
        from trn_agent_boot.trn_boot import _ntff_profile_via_ctypes

        hook = _ntff_profile_via_ctypes("/opt/axon/libaxon_pjrt.so")
    except Exception:
        hook = None
    mod = types.ModuleType("antenv.axon_hooks")
    state = {"hook": hook}
    mod.get_axon_ntff_profile_hook = lambda: state["hook"]
    mod.set_axon_ntff_profile_hook = lambda h: state.__setitem__("hook", h)
    sys.modules["antenv.axon_hooks"] = mod
    import antenv

    antenv.axon_hooks = mod


def kernel(x, w, b, *, trace=False, trace_cores=None):
    if trace:
        _ensure_ntff_hook()
    x = np.asarray(x, dtype=np.float32)
    assert x.shape == (N_NODES, DIM)
    w_pack, bias_col = _host_prep(w, b)

    x_pad = np.zeros((PAD_NODES, DIM), dtype=np.float16)
    x_pad[:N_NODES] = x.astype(np.float16)

    in_maps = []
    for c in range(N_CORES):
        xs = x_pad[c * SHARD:(c + 1) * SHARD]
        xt = np.empty((9, 128, SHARD), dtype=np.float16)
        for bidx, (l, i) in enumerate(BLOCKS):
            off = SEG_OFF_X[l]
            mul, d = IRREPS[l]
            xt[bidx] = xs[:, off + i:off + mul * d:d].T
        in_maps.append({"xt": xt, "w": w_pack, "bias": bias_col})

    if "nc" not in _cache:
        _cache["nc"] = _build()
    res = run_bass_kernel_spmd(
        _cache["nc"], in_maps, list(range(N_CORES)), trace=trace,
        trace_cores=trace_cores,
    )
    _cache["last_result"] = res

    # un-transpose: y_dev[b, v, n] -> y[n, off_l + i + v*d]
    y = np.empty((N_NODES, DIM), dtype=np.float32)
    for c in range(N_CORES):
        ydev = np.asarray(res.results[c]["y"])  # [9, 128, SHARD]
        r0 = c * SHARD
        nrows = min(SHARD, N_NODES - r0)
        if nrows <= 0:
            break
        for bidx, (l, i) in enumerate(BLOCKS):
            off = SEG_OFF_X[l]
            mul, d = IRREPS[l]
            y[r0:r0 + nrows, off + i:off + mul * d:d] = (
                ydev[bidx, :, :nrows].T.astype(np.float32)
            )
    return y


# revision 20
# speedup vs baseline: 1.2268x; 1.2268x over previous
"""Segmented irrep linear (irreps 128x0e+128x1o+128x2e) on 8 TRN2 NeuronCores.

Reference op, per node n (100000 nodes, feature dim 1152):
  y[n, off_l + u*d_l + i] = pw * sum_u' x[n, off_l + u'*d_l + i] * W_l[u', u]
with pw = 128^-0.5, and bias b added on the l=0 (scalar, d=1) output slice.

Strategy (memory-bound: HBM traffic dominates):
  - Data-parallel over nodes: pad to 8 * 12544 rows, one shard per core.
  - All device I/O in fp16 (halves HBM bytes vs fp32; matmul accumulates in
    fp32 PSUM so the end-to-end max error is ~4e-4, far inside the 2e-2
    gate). Host casts x -> fp16 and upcasts y afterwards.
  - Host-side layout prep (cheap, off-device): weights pre-scaled by pw and
    packed [u, (l,v)]; x repacked into nine [u=128, n] planes, one per
    (l, i) = (irrep segment, m-component), then packed BLOCK-MAJOR into a
    flat [128, 9*shard] buffer: each node-block's nine plane slices sit
    contiguously per partition. Every input and output DMA is therefore
    128 descriptors x one contiguous multi-KB run (the DMA queues have a
    ~56 ns/descriptor service floor, so per-partition-interleaved layouts
    with 9 runs/partition/block waste queue time on small blocks). The
    output uses the same flat layout and the host un-packs/transposes it.
  - Device (per core): stream 1024-node blocks. Per 512-node chunk, run
    nine w-stationary matmuls psO[v, n] = W_l^T @ x_(l,i)[u, n] with 512
    moving columns each (one PSUM bank per matmul; long back-to-back PE
    bursts ramp the PE to its 2.4 GHz p-state), add the bias on the l=0
    plane via a per-partition tensor_scalar add, and drain PSUM -> SBUF
    fp16 with DVE (planes 0-3) / ACT (planes 4-8) copies. Input DMAs
    issue on the SP HWDGE ring and output DMAs on the ACT ring so neither
    stream head-of-line-blocks the other.

Measured on trn2 (8 cores, core-0 neuron-profile): fp32 I/O baseline
~321 us; fp16 I/O ~176 us; + w-stationary 512-col matmuls ~165 us
(run-to-run spread under shared-host load is large, 165-210 us).
"""

import numpy as np

import concourse.bass as bass
import concourse.tile as tile
from concourse import bacc, mybir
from concourse.bass_utils import run_bass_kernel_spmd

N_CORES = 8
N_NODES = 100000
DIM = 1152
IRREPS = [(128, 1), (128, 3), (128, 5)]
SEG_OFF_X = [0, 128, 512]
PW = 1.0 / np.sqrt(128.0)

TILE_P = 128
TILES_PER_CORE = 98
SHARD = TILES_PER_CORE * TILE_P  # 12544
PAD_NODES = N_CORES * SHARD  # 100352
NB = 1024  # nodes per DMA block (18KB contiguous fp16 runs per partition)
CHUNK = 512  # moving columns per matmul = one PSUM bank of fp32

# plane order: (l, i) = (irrep segment, m-component), grouped by l so
# consecutive matmuls share the same stationary weight tile
BLOCKS = [(l, i) for l, (mul, d) in enumerate(IRREPS) for i in range(d)]


def _block_sizes(shard=SHARD, nb_size=NB):
    # small blocks first so compute starts early, small blocks last so the
    # compute-paced drain at the end is fine-grained
    head = [128, 128, 256, 512]
    tail = [512, 256, 256, 128, 128]
    rem = shard - sum(head) - sum(tail)
    sizes = list(head)
    while rem > 0:
        m = min(nb_size, rem)
        sizes.append(m)
        rem -= m
    return sizes + tail


_cache = {}


def _build(shard=SHARD, nb_size=NB):
    nc = bacc.Bacc(
        "TRN2", target_bir_lowering=False, debug=False, num_devices=N_CORES
    )
    f32 = mybir.dt.float32
    f16 = mybir.dt.float16
    xt_d = nc.dram_tensor("xt", [128, 9 * shard], f16, kind="ExternalInput")
    w_d = nc.dram_tensor("w", [128, 384], f16, kind="ExternalInput")
    bias_d = nc.dram_tensor("bias", [128, 1], f32, kind="ExternalInput")
    y_d = nc.dram_tensor("y", [128, 9 * shard], f16, kind="ExternalOutput")

    with tile.TileContext(nc) as tc:
        with (
            tc.tile_pool(name="const", bufs=1) as const_pool,
            tc.tile_pool(name="xin", bufs=4) as x_pool,
            tc.tile_pool(name="out", bufs=4) as out_pool,
            tc.tile_pool(name="psO", bufs=8, space=bass.MemorySpace.PSUM) as ps_pool,
        ):
            w_sb = const_pool.tile([128, 384], f16)
            nc.sync.dma_start(w_sb[:], w_d.ap())
            bias_sb = const_pool.tile([128, 1], f32)
            nc.sync.dma_start(bias_sb[:], bias_d.ap())

            n0 = 0
            for nb in _block_sizes(shard, nb_size):
                f0 = 9 * n0  # flat column offset of this block
                x_sb = x_pool.tile([TILE_P, 9 * nb_size], f16, tag="x")
                nc.sync.dma_start(x_sb[:, :9 * nb], xt_d.ap()[:, f0:f0 + 9 * nb])
                out_sb = out_pool.tile([TILE_P, 9 * nb_size], f16, tag="out")

                for c0 in range(0, nb, CHUNK):
                    cw = min(CHUNK, nb - c0)
                    for p in range(9):
                        l = 0 if p == 0 else (1 if p <= 3 else 2)
                        ps = ps_pool.tile([128, CHUNK], f32, tag="ps")
                        nc.tensor.matmul(
                            ps[:, :cw],
                            w_sb[:, l * 128:(l + 1) * 128],
                            x_sb[:, p * nb + c0:p * nb + c0 + cw],
                            start=True, stop=True,
                        )
                        dst = out_sb[:, p * nb + c0:p * nb + c0 + cw]
                        if p == 0:
                            nc.vector.tensor_scalar_add(dst, ps[:, :cw], bias_sb[:])
                        elif p <= 3:
                            nc.vector.tensor_copy(dst, ps[:, :cw])
                        else:
                            nc.scalar.copy(dst, ps[:, :cw])

                # out-DMAs on the ACT HWDGE ring: separate FIFO from the
                # input stream on the SP ring, so a not-yet-ready output
                # can't head-of-line-block input prefetch
                nc.scalar.dma_start(
                    y_d.ap()[:, f0:f0 + 9 * nb], out_sb[:, :9 * nb]
                )
                n0 += nb

    nc.compile()
    return nc


def _host_prep(w, b):
    w = np.asarray(w, dtype=np.float32)
    b = np.asarray(b, dtype=np.float32)
    w_pack = np.empty((128, 384), dtype=np.float16)
    off = 0
    for l, (mul, d) in enumerate(IRREPS):
        W = w[off:off + mul * mul].reshape(mul, mul)  # [u, v]
        w_pack[:, l * 128:(l + 1) * 128] = (PW * W).astype(np.float16)
        off += mul * mul
    bias_col = b.reshape(128, 1).astype(np.float32)
    return w_pack, bias_col


def _ensure_ntff_hook():
    """The agent image's antenv lacks axon_hooks; synthesize it from the
    boot package's ctypes NTFF hook so trace=True works."""
    import sys
    import types

    if "antenv.axon_hooks" in sys.modules:
        return
    try:
        from trn_agent_boot.trn_boot import _ntff_profile_via_ctypes

        hook = _ntff_profile_via_ctypes("/opt/axon/libaxon_pjrt.so")
    except Exception:
        hook = None
    mod = types.ModuleType("antenv.axon_hooks")
    state = {"hook": hook}
    mod.get_axon_ntff_profile_hook = lambda: state["hook"]
    mod.set_axon_ntff_profile_hook = lambda h: state.__setitem__("hook", h)
    sys.modules["antenv.axon_hooks"] = mod
    import antenv

    antenv.axon_hooks = mod


def kernel(x, w, b, *, trace=False, trace_cores=None):
    if trace:
        _ensure_ntff_hook()
    x = np.asarray(x, dtype=np.float32)
    assert x.shape == (N_NODES, DIM)
    w_pack, bias_col = _host_prep(w, b)

    x_pad = np.zeros((PAD_NODES, DIM), dtype=np.float16)
    x_pad[:N_NODES] = x.astype(np.float16)

    sizes = _block_sizes()
    in_maps = []
    for c in range(N_CORES):
        xs = x_pad[c * SHARD:(c + 1) * SHARD]
        planes = np.empty((9, 128, SHARD), dtype=np.float16)
        for bidx, (l, i) in enumerate(BLOCKS):
            off = SEG_OFF_X[l]
            mul, d = IRREPS[l]
            planes[bidx] = xs[:, off + i:off + mul * d:d].T
        # pack block-major: per block, the nine plane slices contiguous
        xt = np.empty((128, 9 * SHARD), dtype=np.float16)
        n0 = 0
        for nb in sizes:
            f0 = 9 * n0
            for p in range(9):
                xt[:, f0 + p * nb:f0 + (p + 1) * nb] = planes[p, :, n0:n0 + nb]
            n0 += nb
        in_maps.append({"xt": xt, "w": w_pack, "bias": bias_col})

    if "nc" not in _cache:
        _cache["nc"] = _build()
    res = run_bass_kernel_spmd(
        _cache["nc"], in_maps, list(range(N_CORES)), trace=trace,
        trace_cores=trace_cores,
    )
    _cache["last_result"] = res

    # un-pack: y_flat[v, 9*n0 + p*nb + j] -> y[n0+j, off_l + i + v*d]
    y = np.empty((N_NODES, DIM), dtype=np.float32)
    for c in range(N_CORES):
        yflat = np.asarray(res.results[c]["y"])  # [128, 9*SHARD] fp16
        r0 = c * SHARD
        if r0 >= N_NODES:
            break
        n0 = 0
        for nb in sizes:
            f0 = 9 * n0
            lo = r0 + n0
            if lo >= N_NODES:
                break
            hi = min(r0 + n0 + nb, N_NODES)
            nrows = hi - lo
            for p, (l, i) in enumerate(BLOCKS):
                off = SEG_OFF_X[l]
                mul, d = IRREPS[l]
                y[lo:hi, off + i:off + mul * d:d] = (
                    yflat[:, f0 + p * nb:f0 + p * nb + nrows].T
                )
            n0 += nb
    return y
